# revision 1
# baseline (speedup 1.0000x reference)
"""Trainium2 Bass kernel for SSTransformer channel-attention block.

Sharding: 8 cores; core c handles sample c//2, row-half c%2 (128 of 256 rows).
Per core, one fused Bass program computes:
  - fused qkv 1x1 conv + depthwise 3x3 (6 PSUM-accumulated matmuls per row over
    a duplicated row-shifted x layout [x; x@+1row] on 128 partitions)
  - Gram matrix of [q;k] (per-head q.k^T dots + squared norms) via PE transpose
  - tiny cross-core AllReduce (core pairs) of the 128x128 Gram
  - on-chip softmax + rel-bias fold into the projection weights
  - (attn@v + proj) and the positional branch dw3x3 -> GELU -> dw3x3, both
    accumulated in PSUM, + proj bias, DMA out.
"""

import sys

sys.path.insert(0, "/opt/trn_rl_repo")

import numpy as np
import ml_dtypes

HEADS = 8
C = 64
CH = 8
B = 4
H = 256
WIMG = 256
WP = 258          # padded row stride (1 zero col each side, shared)
NCORES = 8
ROWS = 128        # output rows per core
VR0 = -2          # v2 first row (relative to slab start)
VROWS = 132       # v2 rows: -2 .. 129
XTOP = 3          # zero-pad rows above image in padded x
XBOT = 4          # below
XROWS = 135       # x rows per core slab: r0-3 .. r0+131
CW = 22           # conv window height (v2 rows per window); 6 windows
NCW = 6
PW = 16           # pe window output rows; 8 windows
NPW = 8
EPS = 1e-12

_BASES = [(-1, -1), (-1, 0), (-1, 1), (1, -1), (1, 0), (1, 1)]  # (dy,dx) of base offsets

_cache = {}


def _build_conv_weights(qkv_w, dw_w):
    """12 lhsT matrices [128,128] f32, flattened to [128, 12*128].

    Matmul m (m<6: pass1 -> [q;k]; m>=6: pass2 -> [v; v@WP]) with base offset
    delta = dy*WP+dx reads partition p<64: x[p] at j+delta, p>=64: x[p-64] at
    j+delta+WP.
    """
    w1 = qkv_w[:, :, 0, 0]  # [192 oc, 64 ic]

    def tapw(oc0, dy, dx):
        # returns [64 ic, 64 oc] = lhsT block for tap (dy,dx), out channels oc0..oc0+64
        blk = w1[oc0 : oc0 + 64]  # [64 oc, 64 ic]
        d = dw_w[oc0 : oc0 + 64, 0, dy + 1, dx + 1]  # [64]
        return (blk * d[:, None]).T.astype(np.float32)

    mats = []
    for dy, dx in _BASES:
        m = np.zeros((128, 128), np.float32)
        # pass1: cols 0:64 = q (oc0=0), 64:128 = k (oc0=64)
        for ci, oc0 in ((0, 0), (64, 64)):
            m[0:64, ci : ci + 64] = tapw(oc0, dy, dx)          # A rows: tap (dy,dx)
            if dy == -1:
                m[64:128, ci : ci + 64] = tapw(oc0, 0, dx)     # B rows: tap (0,dx)
        mats.append(m)
    for dy, dx in _BASES:
        m = np.zeros((128, 128), np.float32)
        # pass2: cols 0:64 = v (oc0=128), 64:128 = v@WP
        m[0:64, 0:64] = tapw(128, dy, dx)
        if dy == -1:
            m[64:128, 0:64] = tapw(128, 0, dx)
            m[64:128, 64:128] = tapw(128, dy, dx)
        else:
            m[0:64, 64:128] = tapw(128, 0, dx)
            m[64:128, 64:128] = tapw(128, dy, dx)
        mats.append(m)
    return np.concatenate(mats, axis=1)  # [128, 12*128]


def _build_dw_weights(pos_w, out_shifted):
    """6 lhsT diag-block matrices for a depthwise 3x3 over [t; t@WP] input.

    out_shifted: if True output is [o; o@WP] (M=128), else just o (M=64).
    Returns [128, 6*M] float32 (cast to bf16 by caller).
    """
    M = 128 if out_shifted else 64

    def dtap(dy, dx):
        return np.diag(pos_w[:, 0, dy + 1, dx + 1]).astype(np.float32)

    mats = []
    for dy, dx in _BASES:
        m = np.zeros((128, M), np.float32)
        m[0:64, 0:64] = dtap(dy, dx)
        if dy == -1:
            m[64:128, 0:64] = dtap(0, dx)
            if out_shifted:
                m[64:128, 64:128] = dtap(dy, dx)
        else:
            if out_shifted:
                m[0:64, 64:128] = dtap(0, dx)
                m[64:128, 64:128] = dtap(dy, dx)
    # NOTE: for dy==+1 and not out_shifted, only A rows are used.
        mats.append(m)
    return np.concatenate(mats, axis=1)


def _build_program(debug=False):
    import concourse.bass as bass
    import concourse.bacc as bacc
    import concourse.mybir as mybir
    from concourse import tile

    dt = mybir.dt
    AF = mybir.ActivationFunctionType
    ALU = mybir.AluOpType
    f32, bf16 = dt.float32, dt.bfloat16

    nc = bacc.Bacc("TRN2", target_bir_lowering=False, debug=False, num_devices=NCORES)

    xp_d = nc.dram_tensor("xp", [C, XROWS * WP], bf16, kind="ExternalInput")
    cw_d = nc.dram_tensor("cw", [128, 12 * 128], bf16, kind="ExternalInput")
    d1_d = nc.dram_tensor("dw1w", [128, 6 * 128], bf16, kind="ExternalInput")
    d2_d = nc.dram_tensor("dw2w", [128, 6 * 64], bf16, kind="ExternalInput")
    idb_d = nc.dram_tensor("idb", [128, 128], bf16, kind="ExternalInput")
    idf_d = nc.dram_tensor("idf", [128, 128], f32, kind="ExternalInput")
    pwT_d = nc.dram_tensor("pwT", [64, 64], f32, kind="ExternalInput")
    wfixT_d = nc.dram_tensor("wfixT", [64, 64], f32, kind="ExternalInput")
    pb_d = nc.dram_tensor("pb", [64, 1], f32, kind="ExternalInput")
    tq_d = nc.dram_tensor("tq", [64, 1], f32, kind="ExternalInput")
    em_d = nc.dram_tensor("emask", [128, 8], f32, kind="ExternalInput")
    blkm_d = nc.dram_tensor("blkm", [64, 64], f32, kind="ExternalInput")
    out_d = nc.dram_tensor("out", [C, ROWS * WIMG], f32, kind="ExternalOutput")
    if debug:
        gdbg_d = nc.dram_tensor("gdbg", [128, 128], f32, kind="ExternalOutput")
        adbg_d = nc.dram_tensor("adbg", [64, 8], f32, kind="ExternalOutput")
        vdbg_d = nc.dram_tensor("vdbg", [128, VROWS * WP], f32, kind="ExternalOutput")

    with tile.TileContext(nc) as tc:
        with (
            tc.tile_pool(name="const", bufs=1) as constp,
            tc.tile_pool(name="big", bufs=1) as bigp,
            tc.tile_pool(name="xwin", bufs=3) as xwp,
            tc.tile_pool(name="rows", bufs=6) as rowp,
            tc.tile_pool(name="glue", bufs=1) as gluep,
            tc.tile_pool(name="gwin", bufs=2) as gwp,
            tc.tile_pool(name="outs", bufs=3) as outp,
            tc.tile_pool(name="psg", bufs=1, space="PSUM") as psgp,
            tc.tile_pool(name="dram", bufs=1, space="DRAM") as dramp,
        ):
            # ---- constants into SBUF ----
            cw = constp.tile([128, 12 * 128], bf16)
            d1w = constp.tile([128, 6 * 128], bf16)
            d2w = constp.tile([128, 6 * 64], bf16)
            idb = constp.tile([128, 128], bf16)
            idf = constp.tile([128, 128], f32)
            pwT = constp.tile([64, 64], f32)
            wfixT = constp.tile([64, 64], f32)
            pb = constp.tile([64, 1], f32)
            tq = constp.tile([64, 1], f32)
            em = constp.tile([128, 8], f32)
            blkm = constp.tile([64, 64], f32)
            for t, d in (
                (cw, cw_d), (d1w, d1_d), (d2w, d2_d), (idb, idb_d), (idf, idf_d),
                (pwT, pwT_d), (wfixT, wfixT_d), (pb, pb_d), (tq, tq_d), (em, em_d),
                (blkm, blkm_d),
            ):
                nc.sync.dma_start(t[:], d.ap())

            # ---- persistent big buffers ----
            v2 = bigp.tile([128, (VROWS + 1) * WP], bf16)  # [v; v@WP], slot y = v row y-2, 1 slack row
            # zero pad columns once (cols 0 and 257 of each row, incl slack)
            v2v = v2[:].rearrange("p (r w) -> p r w", w=WP)
            nc.vector.memset(v2v[:, :, 0:1], 0.0)
            nc.vector.memset(v2v[:, :, 257:258], 0.0)

            G_ps = psgp.tile([128, 128], f32, tag="G")

            # ================= conv + gram phase =================
            gram_first = [True]

            def conv_window(w, psp, psq):
                x2 = xwp.tile([128, 24 * WP], bf16, tag="xwin")
                # copy A: x rows [22w-3, 22w+21) = xp slab rows [22w, 22w+24)
                src0 = 22 * w * WP
                nc.sync.dma_start(x2[0:64, :], xp_d.ap()[:, src0 : src0 + 24 * WP])
                nc.sync.dma_start(x2[64:128, :], xp_d.ap()[:, src0 + WP : src0 + 25 * WP])
                for yv in range(22 * w - 2, 22 * w + 20):
                    slot = yv - (22 * w - 3)  # x-row slot of row yv in window
                    base = slot * WP + 1
                    do_qk = 0 <= yv < ROWS
                    passes = ([(0, True)] if do_qk else []) + [(6, False)]
                    for m0, is_qk in passes:
                        pool = psq if is_qk else psp
                        ps = pool.tile([128, 256], f32, tag="qkps" if is_qk else "vvps")
                        for i, (dy, dx) in enumerate(_BASES):
                            delta = dy * WP + dx
                            nc.tensor.matmul(
                                ps[:],
                                cw[:, 128 * (m0 + i) : 128 * (m0 + i + 1)],
                                x2[:, base + delta : base + delta + 256],
                                start=(i == 0),
                                stop=(i == 5),
                            )
                        if is_qk:
                            qkb = rowp.tile([128, 256], bf16, tag="qkb")
                            nc.scalar.copy(qkb[:], ps[:])
                            qkT = rowp.tile([128, 256], bf16, tag="qkT")
                            for h in range(2):
                                tps = psp.tile([128, 128], bf16, tag="tps")
                                nc.tensor.transpose(tps[:], qkb[:, 128 * h : 128 * h + 128], idb[:])
                                nc.vector.tensor_copy(qkT[:, 128 * h : 128 * h + 128], tps[:])
                            for h in range(2):
                                nc.tensor.matmul(
                                    G_ps[:],
                                    qkT[:, 128 * h : 128 * h + 128],
                                    qkT[:, 128 * h : 128 * h + 128],
                                    start=gram_first[0],
                                    stop=(yv == ROWS - 1 and h == 1),
                                )
                                gram_first[0] = False
                        else:
                            nc.scalar.copy(
                                v2[:, (yv + 2) * WP + 1 : (yv + 2) * WP + 257], ps[:]
                            )

            with (
                tc.tile_pool(name="psA", bufs=2, space="PSUM") as psA,
                tc.tile_pool(name="psQ", bufs=3, space="PSUM") as psQ,
            ):
                for w in range(NCW):
                    conv_window(w, psA, psQ)

            # zero out-of-image v rows (SAME padding for the pe branch)
            for ci, slot in ((3, 0), (4, 1), (5, 129), (6, 130), (7, 131)):
                nc.vector.tensor_scalar(
                    out=v2[:, slot * WP : (slot + 1) * WP],
                    in0=v2[:, slot * WP : (slot + 1) * WP],
                    scalar1=em[:, ci : ci + 1], scalar2=None, op0=ALU.mult,
                )

            # ================= gram allreduce + glue =================
            psB_cm = tc.tile_pool(name="psB", bufs=2, space="PSUM")
            psp = psB_cm.__enter__()
            psO_cm = tc.tile_pool(name="psO", bufs=3, space="PSUM")
            pso = psO_cm.__enter__()
            psC_cm = tc.tile_pool(name="psC", bufs=1, space="PSUM")
            psc = psC_cm.__enter__()
            G_sb = gluep.tile([128, 128], f32)
            nc.scalar.copy(G_sb[:], G_ps[:])
            gin = dramp.tile([128, 128], f32)
            gout = dramp.tile([128, 128], f32)
            nc.sync.dma_start(gin[:], G_sb[:])
            nc.gpsimd.collective_compute(
                "AllReduce",
                mybir.AluOpType.add,
                replica_groups=[[0, 1], [2, 3], [4, 5], [6, 7]],
                ins=[gin[:].opt()],
                outs=[gout[:].opt()],
            )
            G2 = gluep.tile([128, 128], f32)
            nc.sync.dma_start(G2[:], gout[:])
            if debug:
                nc.sync.dma_start(gdbg_d.ap(), G2[:])
                vdbg = gluep.tile([128, VROWS * WP], f32)
                nc.vector.tensor_copy(vdbg[:], v2[:, : VROWS * WP])
                nc.sync.dma_start(vdbg_d.ap(), vdbg[:])

            # diag -> squared norms -> rn = 1/max(sqrt(ssq), eps)
            dd = gluep.tile([128, 128], f32)
            nc.vector.tensor_tensor(out=dd[:], in0=G2[:], in1=idf[:], op=ALU.mult)
            ssq = gluep.tile([128, 1], f32)
            nc.vector.tensor_reduce(ssq[:], dd[:], mybir.AxisListType.X, ALU.add)
            nrm = gluep.tile([128, 1], f32)
            nc.scalar.activation(nrm[:], ssq[:], AF.Sqrt)
            nc.vector.tensor_scalar_max(nrm[:], nrm[:], EPS)
            rn = gluep.tile([128, 1], f32)
            nc.vector.reciprocal(rn[:], nrm[:])
            # Gfull[c,d] = G2[c,d] * rn[c] * rn[d] via scale, transpose, scale, transpose
            Gs = gluep.tile([128, 128], f32)
            nc.vector.tensor_scalar(out=Gs[:], in0=G2[:], scalar1=rn[:], scalar2=None, op0=ALU.mult)
            t1 = psc.tile([128, 128], f32, tag="gt")
            nc.tensor.transpose(t1[:], Gs[:], idf[:])
            GsT = gluep.tile([128, 128], f32)
            nc.vector.tensor_scalar(out=GsT[:], in0=t1[:], scalar1=rn[:], scalar2=None, op0=ALU.mult)
            t2 = psc.tile([128, 128], f32, tag="gt")
            nc.tensor.transpose(t2[:], GsT[:], idf[:])
            Gfull = gluep.tile([128, 128], f32)
            nc.vector.tensor_copy(Gfull[:], t2[:])

            # per-head extraction * temperature -> S [64, 8]
            # masked blockdiag of the q-k quadrant, then strided reduce over groups
            msk = gluep.tile([64, 64], f32)
            nc.vector.tensor_tensor(out=msk[:], in0=Gfull[0:64, 64:128], in1=blkm[:], op=ALU.mult)
            S = gluep.tile([64, 8], f32)
            nc.vector.tensor_reduce(
                S[:], msk[:].rearrange("p (g d) -> p d g", d=8), mybir.AxisListType.X, ALU.add
            )
            nc.vector.tensor_scalar(out=S[:], in0=S[:], scalar1=tq[:], scalar2=None, op0=ALU.mult)
            # softmax along free dim (8)
            nmax = gluep.tile([64, 1], f32)
            nc.vector.tensor_reduce(nmax[:], S[:], mybir.AxisListType.X, ALU.max, negate=True)
            E = gluep.tile([64, 8], f32)
            nc.scalar.activation(E[:], S[:], AF.Exp, bias=nmax[:], scale=1.0)
            Z = gluep.tile([64, 1], f32)
            nc.vector.tensor_reduce(Z[:], E[:], mybir.AxisListType.X, ALU.add)
            rZ = gluep.tile([64, 1], f32)
            nc.vector.reciprocal(rZ[:], Z[:])
            A = gluep.tile([64, 8], f32)
            nc.vector.tensor_scalar(out=A[:], in0=E[:], scalar1=rZ[:], scalar2=None, op0=ALU.mult)
            if debug:
                nc.sync.dma_start(adbg_d.ap(), A[:])
            # blockdiag + fold into projection: WcT = (proj_w @ A_bd)^T + WfixT
            Arep = gluep.tile([64, 64], f32)
            nc.sync.dma_start(Arep[:], A[:].broadcast_to((64, 8, 8)).rearrange("p d g -> p g d"))
            Abd = gluep.tile([64, 64], f32)
            nc.vector.tensor_tensor(out=Abd[:], in0=Arep[:], in1=blkm[:], op=ALU.mult)
            wc_ps = psc.tile([64, 64], f32, tag="wc")
            nc.tensor.matmul(wc_ps[:], Abd[:], pwT[:], start=True, stop=True)
            WcT = gluep.tile([64, 64], bf16)
            nc.vector.tensor_tensor(out=WcT[:], in0=wc_ps[:], in1=wfixT[:], op=ALU.add)

            # ================= pe branch + attn tail =================
            def pe_window(pw):
                gsb = gwp.tile([128, 19 * WP], bf16, tag="gwin")
                gv = gsb[:].rearrange("p (r w) -> p r w", w=WP)
                nc.vector.memset(gv[:, :, 0:1], 0.0)
                nc.vector.memset(gv[:, :, 257:258], 0.0)
                yg0 = PW * pw - 1
                for yg in range(yg0, yg0 + 18):
                    slot = yg - yg0
                    gps = psp.tile([128, 256], f32, tag="gps")
                    vbase = (yg + 2) * WP + 1
                    for i in range(6):
                        dy, dx = _BASES[i]
                        delta = dy * WP + dx
                        nc.tensor.matmul(
                            gps[:],
                            d1w[:, 128 * i : 128 * i + 128],
                            v2[:, vbase + delta : vbase + delta + 256],
                            start=(i == 0),
                            stop=(i == 5),
                        )
                    nc.scalar.activation(
                        gsb[:, slot * WP + 1 : slot * WP + 257], gps[:], AF.Gelu
                    )
                # edge masks (rows outside the image must be zero)
                if pw == 0:
                    nc.vector.tensor_scalar(
                        out=gsb[:, 1:257], in0=gsb[:, 1:257],
                        scalar1=em[:, 0:1], scalar2=None, op0=ALU.mult,
                    )
                if pw == NPW - 1:
                    nc.vector.tensor_scalar(
                        out=gsb[:, 16 * WP + 1 : 16 * WP + 257],
                        in0=gsb[:, 16 * WP + 1 : 16 * WP + 257],
                        scalar1=em[:, 1:2], scalar2=None, op0=ALU.mult,
                    )
                    nc.vector.tensor_scalar(
                        out=gsb[:, 17 * WP + 1 : 17 * WP + 257],
                        in0=gsb[:, 17 * WP + 1 : 17 * WP + 257],
                        scalar1=em[:, 2:3], scalar2=None, op0=ALU.mult,
                    )
                osb = outp.tile([64, PW * 256], f32, tag="osb")
                for yo in range(PW * pw, PW * pw + PW):
                    oslot = yo - PW * pw
                    ops = pso.tile([64, 256], f32, tag="ops")
                    gbase = (yo - yg0) * WP + 1
                    for i in range(6):
                        dy, dx = _BASES[i]
                        delta = dy * WP + dx
                        nc.tensor.matmul(
                            ops[:],
                            d2w[:, 64 * i : 64 * i + 64],
                            gsb[:, gbase + delta : gbase + delta + 256],
                            start=(i == 0),
                            stop=False,
                        )
                    nc.tensor.matmul(
                        ops[:],
                        WcT[:],
                        v2[0:64, (yo + 2) * WP + 1 : (yo + 2) * WP + 257],
                        start=False,
                        stop=True,
                    )
                    nc.scalar.activation(
                        osb[:, oslot * 256 : oslot * 256 + 256], ops[:],
                        AF.Identity, bias=pb[:], scale=1.0,
                    )
                nc.sync.dma_start(
                    out_d.ap()[:, PW * pw * 256 : (PW * pw + PW) * 256], osb[:]
                )

            for pw in range(NPW):
                pe_window(pw)
            psC_cm.__exit__(None, None, None)
            psO_cm.__exit__(None, None, None)
            psB_cm.__exit__(None, None, None)

    nc.compile()
    return nc


def _host_prep(inputs):
    x = np.asarray(inputs["x"], np.float32)
    qkv_w = np.asarray(inputs["qkv_w"], np.float32)
    dw_w = np.asarray(inputs["dw_w"], np.float32)
    proj_w = np.asarray(inputs["proj_w"], np.float32)[:, :, 0, 0]
    proj_b = np.asarray(inputs["proj_b"], np.float32)
    pos1_w = np.asarray(inputs["pos1_w"], np.float32)
    pos2_w = np.asarray(inputs["pos2_w"], np.float32)
    temperature = np.asarray(inputs["temperature"], np.float32).reshape(HEADS)
    rel_bias = np.asarray(inputs["rel_bias"], np.float32)

    cw = _build_conv_weights(qkv_w, dw_w).astype(ml_dtypes.bfloat16)
    d1w = _build_dw_weights(pos1_w, True).astype(ml_dtypes.bfloat16)
    d2w = _build_dw_weights(pos2_w, False).astype(ml_dtypes.bfloat16)
    idb = np.eye(128, dtype=ml_dtypes.bfloat16)
    idf = np.eye(128, dtype=np.float32)
    pwT = np.ascontiguousarray(proj_w.T)  # [m, o]
    ii = np.arange(CH)
    toep = rel_bias[ii[:, None] - ii[None, :] + CH - 1]  # [8, 8]
    wfix = proj_w @ np.kron(np.eye(HEADS, dtype=np.float32), toep)
    wfixT = np.ascontiguousarray(wfix.T.astype(np.float32))
    pb = proj_b.reshape(64, 1)
    tqv = np.repeat(temperature, CH).reshape(64, 1).astype(np.float32)

    blkm_host = np.zeros((64, 64), np.float32)
    for cc in range(64):
        g = cc // CH
        blkm_host[cc, CH * g : CH * g + CH] = 1.0

    # padded x: [B, C, XTOP+H+XBOT, WP]
    xp = np.zeros((B, C, XTOP + H + XBOT, WP), np.float32)
    xp[:, :, XTOP : XTOP + H, 1 : 1 + WIMG] = x.reshape(B, C, H, WIMG)

    in_maps = []
    for core in range(NCORES):
        s, half = core // 2, core % 2
        r0 = half * ROWS
        slab = np.ascontiguousarray(
            xp[s, :, r0 : r0 + XROWS, :].reshape(C, XROWS * WP)
        ).astype(ml_dtypes.bfloat16)
        em = np.ones((128, 8), np.float32)
        if half == 0:
            em[0:64, 0] = 0.0       # g row -1 (A half); B half holds g[0], keep
            em[:, 3] = 0.0          # v2 slot 0 (v[-2] / v[-1])
            em[0:64, 4] = 0.0       # v2 slot 1 A (v[-1]); B holds v[0], keep
        else:
            em[0:64, 2] = 0.0       # g row 128 (A half of slot 17)
            em[64:128, 1] = 0.0     # g row 128 (B half of slot 16)
            em[64:128, 2] = 0.0     # slot 17 B half (g row 129, garbage)
            em[64:128, 5] = 0.0     # v2 slot 129 B (v[128])
            em[:, 6] = 0.0          # v2 slot 130 (v[128] / v[129])
            em[:, 7] = 0.0          # v2 slot 131 (v[129] / v[130])
        in_maps.append(
            {
                "xp": slab, "cw": cw, "dw1w": d1w, "dw2w": d2w, "idb": idb,
                "idf": idf, "pwT": pwT, "wfixT": wfixT, "pb": pb, "tq": tqv,
                "emask": em, "blkm": blkm_host,
            }
        )
    return in_maps


def kernel(**inputs):
    from concourse import bass_utils

    if "prog" not in _cache:
        _cache["prog"] = _build_program()
    nc = _cache["prog"]
    in_maps = _host_prep(inputs)
    res = None
    last = None
    for _attempt in range(3):
        try:
            res = bass_utils.run_bass_kernel_spmd(
                nc, in_maps, core_ids=list(range(NCORES))
            )
            break
        except Exception as e:  # transient device-unrecoverable: reset + retry
            last = e
            try:
                import jax, time as _t

                jax.clear_backends()
                _t.sleep(3)
            except Exception:
                pass
    if res is None:
        raise last
    out = np.empty((B, C, H, WIMG), np.float32)
    for core in range(NCORES):
        s, half = core // 2, core % 2
        r0 = half * ROWS
        out[s, :, r0 : r0 + ROWS, :] = res.results[core]["out"].reshape(C, ROWS, WIMG)
    return out



# revision 2
# speedup vs baseline: 1.1206x; 1.1206x over previous
"""Trainium2 Bass kernel for SSTransformer channel-attention block — v2 "pair".

Sharding: 8 cores; core c handles sample c//2, row-half c%2 (128 of 256 rows).

v2 reduces PE matmul count vs the baseline by producing TWO rows per
accumulation group wherever the output is only 64 channels wide:
  - v conv:  [v(r); v(r+1)] per 6-matmul group (was: v duplicated, 6/row)
  - d1 (pos1 dw3x3): [g(r); g(r+1)] per 6-matmul group (was 6/row)
  - d2 (pos2 dw3x3): [pe(r); pe(r+1)] per 6-matmul group + 1 fused
    blockdiag(WcT) attn@v matmul (was 7/row)
Adjacent-duplicated layouts ([t(y); t(y+1)] per slot) for v and g are built
with one aligned PSUM->SBUF copy (even slots) plus two 64-partition
cross-partition SBUF copies (odd slots) per pair.
qk conv (128 out channels) stays 6 matmuls/row; Gram via PE transpose as
before; tiny core-pair AllReduce of the 128x128 Gram is hidden behind the
d1 phase.
"""

import sys

sys.path.insert(0, "/opt/trn_rl_repo")

import numpy as np
import ml_dtypes

HEADS = 8
C = 64
CH = 8
B = 4
H = 256
WIMG = 256
WP = 258          # padded row stride (1 zero col each side, shared)
NCORES = 8
ROWS = 128        # output rows per core
XTOP = 3          # zero-pad rows above slab start in padded x
XROWS = 135       # x rows per core slab: r0-3 .. r0+131
NV = 133          # v2 dup slots y=-3..129  (slot y = [v(y); v(y+1)]), idx=y+3
NG = 132          # gsb dup slots y=-2..129, idx=y+2
CWIN = 24         # x window rows
NWIN = 6          # x windows (11 pairs each)
EPS = 1e-12

_BASES = [(-1, -1), (-1, 0), (-1, 1), (1, -1), (1, 0), (1, 1)]

_cache = {}


def _tapw(w1, dw_w, oc0, dy, dx):
    blk = w1[oc0 : oc0 + 64]                       # [64 oc, 64 ic]
    d = dw_w[oc0 : oc0 + 64, 0, dy + 1, dx + 1]    # [64]
    return (blk * d[:, None]).T.astype(np.float32)  # [64 ic, 64 oc]


def _build_qk_weights(qkv_w, dw_w):
    """6 lhsT [128,128]: out [q;k] for one row; input [x; x@+1row]."""
    w1 = qkv_w[:, :, 0, 0]
    mats = []
    for dy, dx in _BASES:
        m = np.zeros((128, 128), np.float32)
        for ci, oc0 in ((0, 0), (64, 64)):
            m[0:64, ci : ci + 64] = _tapw(w1, dw_w, oc0, dy, dx)
            if dy == -1:
                m[64:128, ci : ci + 64] = _tapw(w1, dw_w, oc0, 0, dx)
        mats.append(m)
    return np.concatenate(mats, axis=1)


def _build_vv_weights(qkv_w, dw_w):
    """6 lhsT [128,128]: out [v(r); v(r+1)]; input [x; x@+1row]."""
    w1 = qkv_w[:, :, 0, 0]
    mats = []
    for dy, dx in _BASES:
        m = np.zeros((128, 128), np.float32)
        if dy == -1:
            m[0:64, 0:64] = _tapw(w1, dw_w, 128, -1, dx)
            m[64:128, 0:64] = _tapw(w1, dw_w, 128, 0, dx)
            m[64:128, 64:128] = _tapw(w1, dw_w, 128, -1, dx)
        else:
            m[0:64, 0:64] = _tapw(w1, dw_w, 128, 1, dx)
            m[0:64, 64:128] = _tapw(w1, dw_w, 128, 0, dx)
            m[64:128, 64:128] = _tapw(w1, dw_w, 128, 1, dx)
        mats.append(m)
    return np.concatenate(mats, axis=1)


def _build_dpair_weights(pos_w):
    """6 lhsT [128,128]: out [o(r); o(r+1)] from adjacent-dup input slots r-1/r+1."""
    def dtap(dy, dx):
        return np.diag(pos_w[:, 0, dy + 1, dx + 1]).astype(np.float32)

    mats = []
    for dy, dx in _BASES:
        m = np.zeros((128, 128), np.float32)
        if dy == -1:   # reads dup slot r-1 = [t(r-1); t(r)]
            m[0:64, 0:64] = dtap(-1, dx)
            m[64:128, 0:64] = dtap(0, dx)
            m[64:128, 64:128] = dtap(-1, dx)
        else:          # reads dup slot r+1 = [t(r+1); t(r+2)]
            m[0:64, 0:64] = dtap(1, dx)
            m[0:64, 64:128] = dtap(0, dx)
            m[64:128, 64:128] = dtap(1, dx)
        mats.append(m)
    return np.concatenate(mats, axis=1)


def _build_program(debug=False):
    import concourse.bass as bass
    import concourse.bacc as bacc
    import concourse.mybir as mybir
    from concourse import tile

    dt = mybir.dt
    AF = mybir.ActivationFunctionType
    ALU = mybir.AluOpType
    f32, bf16 = dt.float32, dt.bfloat16

    nc = bacc.Bacc("TRN2", target_bir_lowering=False, debug=False, num_devices=NCORES)

    xp_d = nc.dram_tensor("xp", [C, XROWS * WP], bf16, kind="ExternalInput")
    # packed constants: one bf16 block, one f32 block (single DMA each)
    cba_d = nc.dram_tensor("cba", [128, 2 * 768 + 128], bf16, kind="ExternalInput")
    cbb_d = nc.dram_tensor("cbb", [128, 2 * 768], bf16, kind="ExternalInput")
    cf_d = nc.dram_tensor("cf", [128, 264], f32, kind="ExternalInput")
    out_d = nc.dram_tensor("out", [C, ROWS * WIMG], f32, kind="ExternalOutput")

    with tile.TileContext(nc) as tc:
        with (
            tc.tile_pool(name="const", bufs=1) as constp,
            tc.tile_pool(name="big", bufs=1) as bigp,
            tc.tile_pool(name="rows", bufs=6) as rowp,
            tc.tile_pool(name="glue", bufs=1) as gluep,
            tc.tile_pool(name="outs", bufs=3) as outp,
            tc.tile_pool(name="dram", bufs=1, space="DRAM") as dramp,
        ):
            # ---- constants into SBUF (phase-A block now; d1/d2 block later) ----
            cba = constp.tile([128, 2 * 768 + 128], bf16)
            cbb = constp.tile([128, 2 * 768], bf16)
            cf = constp.tile([128, 264], f32)
            nc.sync.dma_start(cba[:], cba_d.ap())
            nc.sync.dma_start(cf[:], cf_d.ap())
            qkw = cba[:, 0:768]
            vvw = cba[:, 768:1536]
            idb = cba[:, 1536:1664]
            d1w = cbb[:, 0:768]
            d2w = cbb[:, 768:1536]
            idf = cf[:, 0:128]
            pwT = cf[0:64, 128:192]
            wfixT = cf[64:128, 128:192]
            blkm = cf[0:64, 192:256]
            pb2 = cf[:, 256:257]
            tq = cf[0:64, 257:258]
            em = cf[:, 258:264]

            # ---- persistent v2 dup slab ----
            v2 = bigp.tile([128, NV * WP], bf16)
            v2v = v2[:].rearrange("p (r w) -> p r w", w=WP)
            nc.vector.memset(v2v[:, :, 0:1], 0.0)
            nc.vector.memset(v2v[:, :, 257:258], 0.0)
            nc.vector.memset(v2[0:64, 0:WP], 0.0)                   # v(-3) A
            nc.vector.memset(v2[64:128, (NV - 1) * WP : NV * WP], 0.0)  # v(130) B

            G_ps_cm = tc.tile_pool(name="psG", bufs=1, space="PSUM")
            psG = G_ps_cm.__enter__()
            G_ps = psG.tile([128, 128], f32, tag="G")

            # ================= phase A: conv (qk + v-pairs) + gram =================
            gram_state = [True]

            def emit_transp_gram(qkb_list, last):
                for qi, qkb in enumerate(qkb_list):
                    for hch in range(2):
                        tps = psT.tile([128, 128], bf16, tag="tps")
                        nc.tensor.transpose(tps[:], qkb[:, 128 * hch : 128 * hch + 128], idb)
                        qkT = rowp.tile([128, 128], bf16, tag="qkT")
                        nc.vector.tensor_copy(qkT[:], tps[:])
                        nc.tensor.matmul(
                            G_ps[:], qkT[:], qkT[:],
                            start=gram_state[0],
                            stop=(last and qi == len(qkb_list) - 1 and hch == 1),
                        )
                        gram_state[0] = False

            with (
                tc.tile_pool(name="xwin", bufs=2) as xwp,
                tc.tile_pool(name="psVV", bufs=2, space="PSUM") as psVV,
                tc.tile_pool(name="psQK", bufs=3, space="PSUM") as psQK,
                tc.tile_pool(name="psT", bufs=2, space="PSUM") as psT,
            ):
                x2w = None
                prev_qkb = []
                for t in range(-1, 65):
                    w = (t + 1) // 11  # window index: pairs [11w-1, 11w+10)
                    if t == 11 * w - 1:
                        x2w = xwp.tile([128, CWIN * WP], bf16, tag="xwin")
                        src0 = 22 * w * WP
                        for ck in range(3):
                            c0, c1 = 8 * ck * WP, 8 * (ck + 1) * WP
                            nc.sync.dma_start(
                                x2w[0:64, c0:c1], xp_d.ap()[:, src0 + c0 : src0 + c1]
                            )
                            nc.sync.dma_start(
                                x2w[64:128, c0:c1],
                                xp_d.ap()[:, src0 + WP + c0 : src0 + WP + c1],
                            )
                    if t == 0:
                        nc.sync.dma_start(cbb[:], cbb_d.ap())
                    r = 2 * t
                    # x2w A-slot j holds x row (22w-3)+j (core-relative)
                    jbase = r - 22 * w + 3
                    # --- vv pair group: out [v(r); v(r+1)] ---
                    vps = psVV.tile([128, 256], f32, tag="vps")
                    for i, (dy, dx) in enumerate(_BASES):
                        off = (jbase + dy) * WP + 1 + dx
                        nc.tensor.matmul(
                            vps[:], vvw[:, 128 * i : 128 * (i + 1)],
                            x2w[:, off : off + 256],
                            start=(i == 0), stop=(i == 5),
                        )
                    # masked copy to v2 even slot idx r+3 (em col 1 on the edge pair)
                    ei = r + 3
                    if t == -1 or t == 64:
                        mc = 1 if t == -1 else 2
                        nc.vector.tensor_scalar(
                            out=v2[:, ei * WP + 1 : ei * WP + 257], in0=vps[:],
                            scalar1=em[:, mc : mc + 1], scalar2=None, op0=ALU.mult,
                        )
                    else:
                        nc.vector.tensor_copy(v2[:, ei * WP + 1 : ei * WP + 257], vps[:])
                    # odd slots: idx r+4 A <- even B ; idx r+2 B <- even A
                    if ei + 1 < NV:
                        nc.vector.tensor_copy(
                            v2[0:64, (ei + 1) * WP + 1 : (ei + 1) * WP + 257],
                            v2[64:128, ei * WP + 1 : ei * WP + 257],
                        )
                    nc.vector.tensor_copy(
                        v2[64:128, (ei - 1) * WP + 1 : (ei - 1) * WP + 257],
                        v2[0:64, ei * WP + 1 : ei * WP + 257],
                    )
                    # --- qk rows r, r+1 ---
                    cur_qkb = []
                    if 0 <= t <= 63:
                        for rr in (r, r + 1):
                            qps = psQK.tile([128, 256], f32, tag="qps")
                            jb = rr - 22 * w + 3
                            for i, (dy, dx) in enumerate(_BASES):
                                off = (jb + dy) * WP + 1 + dx
                                nc.tensor.matmul(
                                    qps[:], qkw[:, 128 * i : 128 * (i + 1)],
                                    x2w[:, off : off + 256],
                                    start=(i == 0), stop=(i == 5),
                                )
                            qkb = rowp.tile([128, 256], bf16, tag="qkb")
                            nc.vector.tensor_copy(qkb[:], qps[:])
                            cur_qkb.append(qkb)
                    # --- transposes+gram for previous pair (pipeline by 1) ---
                    emit_transp_gram(prev_qkb, last=(t == 64))
                    prev_qkb = cur_qkb

            # ================= gram allreduce (hidden behind phase B) =============
            G_sb = gluep.tile([128, 128], f32)
            nc.scalar.copy(G_sb[:], G_ps[:])
            G_ps_cm.__exit__(None, None, None)
            gin = dramp.tile([128, 128], f32)
            gout = dramp.tile([128, 128], f32)
            nc.sync.dma_start(gin[:], G_sb[:])
            nc.gpsimd.collective_compute(
                "AllReduce",
                mybir.AluOpType.add,
                replica_groups=[[0, 1], [2, 3], [4, 5], [6, 7]],
                ins=[gin[:].opt()],
                outs=[gout[:].opt()],
            )
            G2 = gluep.tile([128, 128], f32)
            nc.sync.dma_start(G2[:], gout[:])

            # ================= phase B: d1 pairs -> gsb dup =======================
            psO_cm = tc.tile_pool(name="psO", bufs=3, space="PSUM")
            psO = psO_cm.__enter__()
            psC_cm = tc.tile_pool(name="psC", bufs=1, space="PSUM")
            psc = psC_cm.__enter__()

            gsb = bigp.tile([128, NG * WP], bf16)
            gv = gsb[:].rearrange("p (r w) -> p r w", w=WP)
            nc.vector.memset(gv[:, :, 0:1], 0.0)
            nc.vector.memset(gv[:, :, 257:258], 0.0)
            nc.vector.memset(gsb[64:128, (NG - 1) * WP : NG * WP], 0.0)  # g(130) B

            for sg in range(-1, 65):
                r = 2 * sg
                gps = psO.tile([128, 256], f32, tag="gps")
                for i, (dy, dx) in enumerate(_BASES):
                    off = (r + dy + 3) * WP + 1 + dx
                    nc.tensor.matmul(
                        gps[:], d1w[:, 128 * i : 128 * (i + 1)],
                        v2[:, off : off + 256],
                        start=(i == 0), stop=(i == 5),
                    )
                mc = 4 if sg == -1 else (5 if sg == 64 else 3)
                ei = r + 2
                nc.scalar.activation(
                    gsb[:, ei * WP + 1 : ei * WP + 257], gps[:],
                    AF.Gelu, scale=em[:, mc : mc + 1],
                )
                if ei + 1 < NG:
                    nc.vector.tensor_copy(
                        gsb[0:64, (ei + 1) * WP + 1 : (ei + 1) * WP + 257],
                        gsb[64:128, ei * WP + 1 : ei * WP + 257],
                    )
                if ei - 1 >= 0:
                    nc.vector.tensor_copy(
                        gsb[64:128, (ei - 1) * WP + 1 : (ei - 1) * WP + 257],
                        gsb[0:64, ei * WP + 1 : ei * WP + 257],
                    )

            # ================= glue: norms, softmax, fold into WcT ================
            dd = gluep.tile([128, 128], f32)
            nc.vector.tensor_tensor(out=dd[:], in0=G2[:], in1=idf, op=ALU.mult)
            ssq = gluep.tile([128, 1], f32)
            nc.vector.tensor_reduce(ssq[:], dd[:], mybir.AxisListType.X, ALU.add)
            nrm = gluep.tile([128, 1], f32)
            nc.scalar.activation(nrm[:], ssq[:], AF.Sqrt)
            nc.vector.tensor_scalar_max(nrm[:], nrm[:], EPS)
            rn = gluep.tile([128, 1], f32)
            nc.vector.reciprocal(rn[:], nrm[:])
            Gs = gluep.tile([128, 128], f32)
            nc.vector.tensor_scalar(out=Gs[:], in0=G2[:], scalar1=rn[:], scalar2=None, op0=ALU.mult)
            t1 = psc.tile([128, 128], f32, tag="gt")
            nc.tensor.transpose(t1[:], Gs[:], idf)
            GsT = gluep.tile([128, 128], f32)
            nc.vector.tensor_scalar(out=GsT[:], in0=t1[:], scalar1=rn[:], scalar2=None, op0=ALU.mult)
            # S [64,8]: per-head block of the q-k quadrant (rows q, cols k)
            # GsT[c,d] = Gfull[d,c]; we need Gfull[0:64, 64:128] = GsT[64:128, 0:64]^T
            t2 = psc.tile([128, 128], f32, tag="gt")
            nc.tensor.transpose(t2[:], GsT[:], idf)
            Gfull = gluep.tile([128, 128], f32)
            nc.vector.tensor_copy(Gfull[:], t2[:])
            msk = gluep.tile([64, 64], f32)
            nc.vector.tensor_tensor(out=msk[:], in0=Gfull[0:64, 64:128], in1=blkm, op=ALU.mult)
            S = gluep.tile([64, 8], f32)
            nc.vector.tensor_reduce(
                S[:], msk[:].rearrange("p (g d) -> p d g", d=8), mybir.AxisListType.X, ALU.add
            )
            nc.vector.tensor_scalar(out=S[:], in0=S[:], scalar1=tq, scalar2=None, op0=ALU.mult)
            nmax = gluep.tile([64, 1], f32)
            nc.vector.tensor_reduce(nmax[:], S[:], mybir.AxisListType.X, ALU.max, negate=True)
            E = gluep.tile([64, 8], f32)
            nc.scalar.activation(E[:], S[:], AF.Exp, bias=nmax[:], scale=1.0)
            Z = gluep.tile([64, 1], f32)
            nc.vector.tensor_reduce(Z[:], E[:], mybir.AxisListType.X, ALU.add)
            rZ = gluep.tile([64, 1], f32)
            nc.vector.reciprocal(rZ[:], Z[:])
            A = gluep.tile([64, 8], f32)
            nc.vector.tensor_scalar(out=A[:], in0=E[:], scalar1=rZ[:], scalar2=None, op0=ALU.mult)
            Arep = gluep.tile([64, 64], f32)
            nc.sync.dma_start(Arep[:], A[:].broadcast_to((64, 8, 8)).rearrange("p d g -> p g d"))
            Abd = gluep.tile([64, 64], f32)
            nc.vector.tensor_tensor(out=Abd[:], in0=Arep[:], in1=blkm, op=ALU.mult)
            wc_ps = psc.tile([64, 64], f32, tag="wc")
            nc.tensor.matmul(wc_ps[:], Abd[:], pwT, start=True, stop=True)
            WcT = gluep.tile([64, 64], bf16)
            nc.vector.tensor_tensor(out=WcT[:], in0=wc_ps[:], in1=wfixT, op=ALU.add)
            Wc2 = gluep.tile([128, 128], bf16)
            nc.vector.memset(Wc2[:], 0.0)
            nc.vector.tensor_copy(Wc2[0:64, 0:64], WcT[:])
            nc.vector.tensor_copy(Wc2[64:128, 64:128], WcT[:])

            # ================= phase C: d2 pairs + fused attn@v + proj ============
            for jw in range(16):
                osb = outp.tile([128, 4 * 256], f32, tag="osb")
                for pj in range(4):
                    r = 8 * jw + 2 * pj
                    ops = psO.tile([128, 256], f32, tag="ops")
                    for i, (dy, dx) in enumerate(_BASES):
                        off = (r + dy + 2) * WP + 1 + dx
                        nc.tensor.matmul(
                            ops[:], d2w[:, 128 * i : 128 * (i + 1)],
                            gsb[:, off : off + 256],
                            start=(i == 0), stop=False,
                        )
                    nc.tensor.matmul(
                        ops[:], Wc2[:],
                        v2[:, (r + 3) * WP + 1 : (r + 3) * WP + 257],
                        start=False, stop=True,
                    )
                    nc.scalar.activation(
                        osb[:, pj * 256 : pj * 256 + 256], ops[:],
                        AF.Identity, bias=pb2, scale=1.0,
                    )
                # out_d view [64, 64 pairblocks, 512]: pair rw -> rows 2rw | 2rw+1
                ov = out_d.ap().rearrange("p (rw tw) -> p rw tw", tw=512)
                ob = osb[:].rearrange("p (pj w) -> p pj w", w=256)
                nc.sync.dma_start(ov[:, 4 * jw : 4 * jw + 4, 0:256], ob[0:64])
                nc.sync.dma_start(ov[:, 4 * jw : 4 * jw + 4, 256:512], ob[64:128])

            psC_cm.__exit__(None, None, None)
            psO_cm.__exit__(None, None, None)

    nc.compile()
    return nc


def _host_prep(inputs):
    x = np.asarray(inputs["x"], np.float32)
    qkv_w = np.asarray(inputs["qkv_w"], np.float32)
    dw_w = np.asarray(inputs["dw_w"], np.float32)
    proj_w = np.asarray(inputs["proj_w"], np.float32)[:, :, 0, 0]
    proj_b = np.asarray(inputs["proj_b"], np.float32)
    pos1_w = np.asarray(inputs["pos1_w"], np.float32)
    pos2_w = np.asarray(inputs["pos2_w"], np.float32)
    temperature = np.asarray(inputs["temperature"], np.float32).reshape(HEADS)
    rel_bias = np.asarray(inputs["rel_bias"], np.float32)

    qkw = _build_qk_weights(qkv_w, dw_w).astype(ml_dtypes.bfloat16)
    vvw = _build_vv_weights(qkv_w, dw_w).astype(ml_dtypes.bfloat16)
    d1w = _build_dpair_weights(pos1_w).astype(ml_dtypes.bfloat16)
    d2w = _build_dpair_weights(pos2_w).astype(ml_dtypes.bfloat16)
    idb = np.eye(128, dtype=ml_dtypes.bfloat16)
    cba = np.concatenate([qkw, vvw, idb], axis=1)
    cbb = np.concatenate([d1w, d2w], axis=1)
    idf = np.eye(128, dtype=np.float32)
    pwT = np.ascontiguousarray(proj_w.T)
    ii = np.arange(CH)
    toep = rel_bias[ii[:, None] - ii[None, :] + CH - 1]
    wfix = proj_w @ np.kron(np.eye(HEADS, dtype=np.float32), toep)
    wfixT = np.ascontiguousarray(wfix.T.astype(np.float32))
    pb2 = np.concatenate([proj_b, proj_b]).reshape(128, 1)
    tqv = np.repeat(temperature, CH).reshape(64, 1).astype(np.float32)

    blkm_host = np.zeros((64, 64), np.float32)
    for cc in range(64):
        g = cc // CH
        blkm_host[cc, CH * g : CH * g + CH] = 1.0

    xp = np.zeros((B, C, XTOP + H + 4, WP), np.float32)
    xp[:, :, XTOP : XTOP + H, 1 : 1 + WIMG] = x.reshape(B, C, H, WIMG)

    in_maps = []
    for core in range(NCORES):
        s, half = core // 2, core % 2
        r0 = half * ROWS
        slab = np.ascontiguousarray(
            xp[s, :, r0 : r0 + XROWS, :].reshape(C, XROWS * WP)
        ).astype(ml_dtypes.bfloat16)
        # em cols: 0=ones, 1=vv mask@t=-1, 2=vv mask@t=64,
        #          3=ones, 4=g mask@sg=-1, 5=g mask@sg=64
        em = np.ones((128, 6), np.float32)
        if half == 0:
            em[:, 1] = 0.0   # v(-2), v(-1) out of image
            em[:, 4] = 0.0   # g(-2), g(-1) zero (SAME pad at image top)
        else:
            em[:, 2] = 0.0   # v(128), v(129) out of image
            em[:, 5] = 0.0   # g(128), g(129) zero (SAME pad at image bottom)
        cf = np.zeros((128, 264), np.float32)
        cf[:, 0:128] = idf
        cf[0:64, 128:192] = pwT
        cf[64:128, 128:192] = wfixT
        cf[0:64, 192:256] = blkm_host
        cf[:, 256:257] = pb2
        cf[0:64, 257:258] = tqv
        cf[:, 258:264] = em
        in_maps.append({"xp": slab, "cba": cba, "cbb": cbb, "cf": cf})
    return in_maps


def kernel(**inputs):
    from concourse import bass_utils

    if "prog" not in _cache:
        _cache["prog"] = _build_program()
    nc = _cache["prog"]
    in_maps = _host_prep(inputs)
    res = None
    last = None
    for _attempt in range(3):
        try:
            res = bass_utils.run_bass_kernel_spmd(
                nc, in_maps, core_ids=list(range(NCORES))
            )
            break
        except Exception as e:  # transient device-unrecoverable: reset + retry
            last = e
            try:
                import jax, time as _t

                jax.clear_backends()
                _t.sleep(3)
            except Exception:
                pass
    if res is None:
        raise last
    out = np.empty((B, C, H, WIMG), np.float32)
    for core in range(NCORES):
        s, half = core // 2, core % 2
        r0 = half * ROWS
        out[s, :, r0 : r0 + ROWS, :] = res.results[core]["out"].reshape(C, ROWS, WIMG)
    return out
